# revision 32
# baseline (speedup 1.0000x reference)
"""Binarized 3x3 conv (GeneralConv2d) on 8 NeuronCores.

y[b,o,h,w] = mean_abs(w[o]) * sum_{c,kh,kw} sign(w[o,c,kh,kw]) * x[b,c,h+kh-1,w+kw-1]

The module init is torch.rand(...)*1e-3, so every weight is >= 0 and
sign(w) == +1 everywhere.  The conv then collapses to

    y[b,o] = scale_o * box3x3( sum_c x[b,c] )

which is memory-bound, not compute-bound: per core (4 images, data-parallel
over batch) the channel sum runs as all-ones 128x128 matmuls, the 3x3 box
filter as two separable shifted-add passes on DVE over a flat layout with
1-col / 1-row zero margins (plus two strided edge-wrap fixes per image),
and the per-out-channel scale as a per-partition tensor_scalar on Act.

The schedule targets the 360 GB/s DMA roofline: x streams in as bf16
(cast DMAs are charged at destination width), w as fp8e5 (its [0, 1e-3)
range lives in e5m2 denormals, plenty for a 2304-wide mean), y out as the
mandated f32.  Per-image stages are split in halves pipelined across
PE -> Act -> DVE so the output-DMA stream starts before the input stream
finishes and never starves; the w row-mean runs as three bf16 tensor_add
folds plus a short reduce, emitted inside image 0's DVE stream where the
engine is otherwise waiting.

A general dense-conv path (18 shifted GEMMs per output chunk) is kept as a
fallback and used only if any weight is negative.
"""

import numpy as np

from contextlib import ExitStack

import concourse.bass as bass
import concourse.mybir as mybir
from concourse import bacc
import concourse.tile as tile
from concourse.masks import make_identity

dt = mybir.dt
OUT_C = 256
IN_C = 256
KH = KW = 3
KK = KH * KW           # 9
CKK = IN_C * KK        # 2304
P = 128
CC = IN_C // P         # 2 in-channel chunks
OO = OUT_C // P        # 2 out-channel chunks
QC = CKK // CC         # 1152 columns per (oo,cc) quarter


# ---------------------------------------------------------------------------
# Fast path: all weights >= 0  ->  y = scale_o * box3x3(channel_sum(x))
# ---------------------------------------------------------------------------

def _build_boxsum_nc(imgs: int, H: int, W: int, evict=("act", "act")):
    HW = H * W             # 3136
    HALF = HW // 2         # 1568 = 3*512 + 32 (psum groups bank-aligned)
    # Stage split points (all in output rows) chosen so each downstream
    # half-stage only depends on the matching upstream half plus margins:
    # evict halves at chx col 1568; W-pass halves at R row 27 (flat 1512);
    # H/scale/store halves at U row 26 (flat 1456).
    WSPL = 27 * W          # 1512
    USPL = 26 * W          # 1456
    nc = bacc.Bacc("TRN2", target_bir_lowering=False, debug=False,
                   enable_asserts=False, num_devices=8)
    x = nc.declare_dram_parameter("x", [imgs, IN_C, H, W], dt.float32, isOutput=False)
    w = nc.declare_dram_parameter("w", [OUT_C * CKK, 1], dt.float32, isOutput=False)
    y = nc.declare_dram_parameter("y", [imgs, OUT_C, H, W], dt.float32, isOutput=True)
    w2d = w.rearrange("(o r) one -> o (r one)", r=CKK)   # [256, 2304]

    with tile.TileContext(nc) as tc, ExitStack() as ctx:
        EVICT = [
            {"act": nc.scalar.copy,
             "dve": (lambda out, in_: nc.vector.tensor_copy(out=out, in_=in_))}[e]
            for e in evict]
        consts = ctx.enter_context(tc.tile_pool(name="consts", bufs=1))
        ones = consts.tile([P, P], dt.bfloat16)
        nc.vector.memset(ones, 1.0)

        wprep = ctx.enter_context(tc.tile_pool(name="wprep", bufs=1))
        w_sb = wprep.tile([P, OO, CKK], dt.float8e5)
        wf = wprep.tile([P, OO, CKK // 2], dt.bfloat16)
        scale_sb = wprep.tile([P, OO], dt.float32)

        xp = ctx.enter_context(tc.tile_pool(name="xtiles", bufs=imgs * CC))
        chp = ctx.enter_context(tc.tile_pool(name="chx", bufs=3))
        rp = ctx.enter_context(tc.tile_pool(name="rrow", bufs=3))
        up = ctx.enter_context(tc.tile_pool(name="usum", bufs=3))
        stp = ctx.enter_context(tc.tile_pool(name="ostage", bufs=8))
        pp = ctx.enter_context(tc.tile_pool(name="psum", bufs=2, space="PSUM"))

        xt = {}

        def load_x(img, split=False):
            # img0 loads in halves so its first matmuls start ~1us sooner;
            # later images load whole (same descriptor count per byte but
            # half as many SWDGE generations, which pace the input stream).
            xf = x.rearrange("i c h w -> i c (h w)")
            for cc in range(CC):
                if (img, cc) not in xt:
                    xt[(img, cc)] = xp.tile([P, HW], dt.bfloat16,
                                            name=f"x_{img}_{cc}", bufs=1)
            chunks = ((0, HALF), (HALF, HW)) if split else ((0, HW),)
            for (b0, b1) in chunks:
                for cc in range(CC):
                    nc.gpsimd.dma_start(
                        out=xt[(img, cc)][:, b0:b1],
                        in_=xf[img, cc * P:(cc + 1) * P, b0:b1])

        def load_w():
            for oo in range(OO):
                nc.gpsimd.dma_start(out=w_sb[:, oo],
                                    in_=w2d[oo * P:(oo + 1) * P, :])
            # Two-stage reduce: 64-element partials first so the result is
            # exact-ish even if the reducer accumulates at input precision.
        def reduce_w(oo):
            # Three tensor_add folds shrink 2304 -> 288 columns, then one
            # f32-accumulated reduce and the 1/CKK scaling finish the row
            # mean.  w >= 0 on this path, so no abs needed before summing.
            h1, h2, h3 = CKK // 2, CKK // 4, CKK // 8
            nc.vector.tensor_add(wf[:, oo], w_sb[:, oo, 0:h1],
                                 w_sb[:, oo, h1:CKK])
            nc.vector.tensor_add(wf[:, oo, 0:h2], wf[:, oo, 0:h2],
                                 wf[:, oo, h2:h1])
            nc.vector.tensor_add(wf[:, oo, 0:h3], wf[:, oo, 0:h3],
                                 wf[:, oo, h3:h2])
            nc.vector.tensor_reduce(
                out=scale_sb[:, oo:oo + 1], in_=wf[:, oo, 0:h3],
                axis=mybir.AxisListType.X,
                op=mybir.AluOpType.add, apply_absolute_value=False)
            nc.vector.tensor_scalar_mul(scale_sb[:, oo:oo + 1],
                                        scale_sb[:, oo:oo + 1], 1.0 / CKK)

        def conv_img(img, mid_w=None, pre_store=None):
            # chx with a 1-col zero margin on each side: chm[:, 1+i] = chx[i]
            chm = chp.tile([P, HW + 2], dt.bfloat16)
            nc.gpsimd.memset(chm[:, 0:1], 0.0)
            nc.gpsimd.memset(chm[:, HW + 1:HW + 2], 0.0)
            for half in range(2):
                ps = pp.tile([P, HALF], dt.float32)
                base = half * HALF
                g0 = 0
                while g0 < HALF:           # 512-col groups stay in one bank
                    g1 = min(g0 + 512, HALF)
                    for cc in range(CC):
                        nc.tensor.matmul(
                            ps[:, g0:g1], lhsT=ones,
                            rhs=xt[(img, cc)][:, base + g0:base + g1],
                            start=(cc == 0), stop=(cc == CC - 1))
                    g0 = g1
                EVICT[half](out=chm[:, 1 + base:1 + base + HALF], in_=ps)

            # W pass: rm row r (r=1..56) holds R[h=r-1]; rows 0,57 stay zero.
            # R[i] = chx[i-1] + chx[i] + chx[i+1] over the flat index, with
            # the row-edge wraparound fixed below.  Split at R row 27 so the
            # first half only reads the first evict.
            rm = rp.tile([P, (H + 2) * W], dt.bfloat16)
            nc.gpsimd.memset(rm[:, 0:W], 0.0)
            nc.gpsimd.memset(rm[:, (H + 1) * W:(H + 2) * W], 0.0)
            rv = rm.rearrange("p (r w) -> p r w", w=W)
            chv = chm[:, 1:1 + HW].rearrange("p (h w) -> p h w", w=W)
            for hi, (i0, i1) in enumerate(((0, WSPL), (WSPL, HW))):
                o = W + i0
                n = i1 - i0
                nc.vector.tensor_add(rm[:, o:o + n], chm[:, i0:i0 + n],
                                     chm[:, i0 + 1:i0 + n + 1])
                nc.vector.tensor_add(rm[:, o:o + n], rm[:, o:o + n],
                                     chm[:, i0 + 2:i0 + n + 2])

            #   R[h,0]  wrongly includes chx[h-1,55]; R[h,55] includes chx[h+1,0].
            if mid_w:
                mid_w()
            # Pool engine is free of DMA-descriptor work for the later
            # images, so the tiny strided edge fixes go there to keep DVE on
            # the big adds; early images keep them on DVE so Pool never
            # stalls the input-DMA descriptor stream.
            fx = nc.gpsimd if img >= 2 else nc.vector
            fx.tensor_sub(rv[:, 2:28, 0:1], rv[:, 2:28, 0:1],
                          chv[:, 0:26, W - 1:W])
            fx.tensor_sub(rv[:, 1:27, W - 1:W], rv[:, 1:27, W - 1:W],
                          chv[:, 1:27, 0:1])
            fx.tensor_sub(rv[:, 28:H + 1, 0:1], rv[:, 28:H + 1, 0:1],
                          chv[:, 26:H - 1, W - 1:W])
            fx.tensor_sub(rv[:, 27:H, W - 1:W], rv[:, 27:H, W - 1:W],
                          chv[:, 27:H, 0:1])

            # H pass: U[h] = R[h-1] + R[h] + R[h+1]; split at U row 26 so the
            # first half only reads rm below the W split (plus margins/fixes).
            # The last image gets a finer first slice so its first store (the
            # tail of the output-DMA stream) is ready as early as possible.
            u = up.tile([P, HW], dt.bfloat16)
            parts = ((0, USPL), (USPL, HW))
            if img == imgs - 1:
                parts = ((0, 13 * W), (13 * W, USPL), (USPL, HW))
            for pi, (i0, i1) in enumerate(parts):
                n = i1 - i0
                nc.vector.tensor_add(u[:, i0:i1], rm[:, i0:i0 + n],
                                     rm[:, i0 + W:i0 + W + n])
                nc.vector.tensor_add(u[:, i0:i1], u[:, i0:i1],
                                     rm[:, i0 + 2 * W:i0 + 2 * W + n])
                if pi == 0 and pre_store:
                    pre_store()
            for (i0, i1) in parts:
                r0, r1 = i0 // W, i1 // W
                for oo in range(OO):
                    st = stp.tile([P, i1 - i0], dt.float32)
                    nc.scalar.mul(st, u[:, i0:i1], scale_sb[:, oo:oo + 1])
                    nc.sync.dma_start(
                        out=y[img, oo * P:(oo + 1) * P, r0:r1].rearrange(
                            "c h w -> c (h w)"),
                        in_=st)

        load_x(0, split=True)
        load_w()
        load_x(1)
        conv_img(0, mid_w=lambda: reduce_w(0),
                 pre_store=lambda: reduce_w(1))
        load_x(2)
        conv_img(1)
        load_x(3)
        conv_img(2)
        conv_img(3)
    nc.compile()
    return nc


# ---------------------------------------------------------------------------
# Fallback: general binarized conv (18 shifted GEMMs), used if any w < 0
# ---------------------------------------------------------------------------

def _build_conv_nc(imgs: int, H: int, W: int, hchunk: int, psum_bufs: int = 7,
                  ostage_bufs: int = 4, gsz: int = 4, tp_bufs: int = 1):
    assert H % hchunk == 0
    nch = H // hchunk
    Hp, Wp = H + 2, W + 2
    nc = bacc.Bacc("TRN2", target_bir_lowering=False, debug=False,
                   enable_asserts=False, num_devices=8)
    x = nc.declare_dram_parameter("x", [imgs, IN_C, H, W], dt.float32, isOutput=False)
    w = nc.declare_dram_parameter("w", [OUT_C * CKK, 1], dt.float32, isOutput=False)
    y = nc.declare_dram_parameter("y", [imgs, OUT_C, H, W], dt.float32, isOutput=True)

    w2d = w.rearrange("(o r) one -> o (r one)", r=CKK)   # [256, 2304]

    with tile.TileContext(nc) as tc, ExitStack() as ctx:
        consts = ctx.enter_context(tc.tile_pool(name="consts", bufs=1))
        ident = consts.tile([P, P], dt.bfloat16)
        make_identity(nc, ident)
        zrow = consts.tile([P, 2 * Wp], dt.bfloat16)
        nc.vector.memset(zrow, 0.0)

        wprep = ctx.enter_context(tc.tile_pool(name="wprep", bufs=1))
        w_sb = wprep.tile([P, OO, CKK], dt.float32)
        sgn_sb = wprep.tile([P, OO, CKK], dt.bfloat16)
        scale_sb = wprep.tile([P, OO], dt.float32)
        sgn_v = sgn_sb.rearrange("p oo (c k) -> p oo c k", k=KK)

        tpool = ctx.enter_context(tc.tile_pool(name="tpsum", bufs=tp_bufs, space="PSUM"))
        wtp = ctx.enter_context(tc.tile_pool(name="wtiles", bufs=OO * CC * KK))
        xp = ctx.enter_context(tc.tile_pool(name="xtiles", bufs=imgs * CC))

        xt = {}

        def load_x(img):
            for cc in range(CC):
                t = xp.tile([P, Hp, Wp], dt.bfloat16)
                tf = t.rearrange("p h w -> p (h w)")
                nc.scalar.copy(tf[:, 0:Wp], zrow[:, 0:Wp])
                nc.scalar.copy(tf[:, (Hp - 1) * Wp:Hp * Wp], zrow[:, 0:Wp])
                mid = tf[:, Wp - 1:Wp - 1 + (Hp - 1) * Wp].rearrange(
                    "p (h w) -> p h w", w=Wp)[:, :, 0:2]
                nc.scalar.copy(mid, zrow[:, 0:2 * (Hp - 1)].rearrange(
                    "p (h w) -> p h w", w=2))
                h2 = H // 2
                nc.gpsimd.dma_start(out=t[:, 1:h2 + 1, 1:W + 1],
                                    in_=x[img, cc * P:(cc + 1) * P, 0:h2])
                nc.gpsimd.dma_start(out=t[:, h2 + 1:H + 1, 1:W + 1],
                                    in_=x[img, cc * P:(cc + 1) * P, h2:H])
                xt[(img, cc)] = t

        wt = {}

        def prep_w_quarter(oo, cc):
            q2 = QC // 2
            for h in range(2):
                nc.sync.dma_start(
                    out=w_sb[:, oo, cc * QC + h * q2:cc * QC + (h + 1) * q2],
                    in_=w2d[oo * P:(oo + 1) * P,
                            cc * QC + h * q2:cc * QC + (h + 1) * q2])
            nc.vector.tensor_scalar(
                out=sgn_sb[:, oo, cc * QC:(cc + 1) * QC],
                in0=w_sb[:, oo, cc * QC:(cc + 1) * QC],
                scalar1=0.0, scalar2=2.0,
                op0=mybir.AluOpType.is_ge, op1=mybir.AluOpType.mult)
            nc.vector.tensor_scalar_add(
                sgn_sb[:, oo, cc * QC:(cc + 1) * QC],
                sgn_sb[:, oo, cc * QC:(cc + 1) * QC], -1.0)
            for k in range(KK):
                tp = tpool.tile([P, P], dt.bfloat16)
                nc.tensor.transpose(tp, sgn_v[:, oo, cc * P:(cc + 1) * P, k], ident)
                t = wtp.tile([P, P], dt.bfloat16)
                nc.vector.tensor_copy(out=t, in_=tp)
                wt[(oo, cc, k)] = t

        def reduce_scale(oo):
            nc.vector.tensor_reduce(
                out=scale_sb[:, oo:oo + 1], in_=w_sb[:, oo, :],
                axis=mybir.AxisListType.X,
                op=mybir.AluOpType.add, apply_absolute_value=True)
            nc.vector.tensor_scalar_mul(
                scale_sb[:, oo:oo + 1], scale_sb[:, oo:oo + 1], 1.0 / CKK)

        pp = ctx.enter_context(tc.tile_pool(name="psum", bufs=psum_bufs, space="PSUM"))
        op = ctx.enter_context(tc.tile_pool(name="ostage", bufs=ostage_bufs))

        def mm(ps, img, oo, cc, ih, k, n):
            ki, kj = divmod(k, KW)
            rhs = xt[(img, cc)][
                :, ih * hchunk + ki: ih * hchunk + ki + hchunk, kj: kj + W]
            nc.tensor.matmul(ps, lhsT=wt[(oo, cc, k)], rhs=rhs,
                             start=(n == 0), stop=(n == CC * KK - 1))

        def conv_a(img, oo, tiles):
            group = {}
            for ih in tiles:
                ps = pp.tile([P, hchunk * W], dt.float32,
                             name=f"ps_{img}_{oo}_{ih}", tag="ps")
                group[ih] = ps
                for k in range(KK):
                    mm(ps, img, oo, 0, ih, k, n=k)
            return group

        def conv_b(img, oo, group):
            for ih, ps in group.items():
                for k in range(KK):
                    mm(ps, img, oo, 1, ih, k, n=KK + k)
                st = op.tile([P, hchunk * W], dt.float32,
                             name=f"st_{img}_{oo}_{ih}", tag="st")
                nc.scalar.mul(st, ps, scale_sb[:, oo:oo + 1])
                nc.sync.dma_start(
                    out=y[img, oo * P:(oo + 1) * P,
                          ih * hchunk:(ih + 1) * hchunk, :],
                    in_=st)

        def conv(img, oo, skip=0):
            for g0 in range(skip, nch, gsz):
                tiles = list(range(g0, min(g0 + gsz, nch)))
                conv_b(img, oo, conv_a(img, oo, tiles))

        load_x(0)
        prep_w_quarter(0, 0)
        a1 = conv_a(0, 0, list(range(min(gsz, nch))))
        prep_w_quarter(0, 1)
        reduce_scale(0)
        if imgs > 1:
            load_x(1)
        conv_b(0, 0, a1)
        prep_w_quarter(1, 0)
        if nch > gsz:
            a2 = conv_a(0, 0, list(range(gsz, min(2 * gsz, nch))))
            prep_w_quarter(1, 1)
            reduce_scale(1)
            conv_b(0, 0, a2)
            conv(0, 0, skip=2 * gsz)
        else:
            prep_w_quarter(1, 1)
            reduce_scale(1)
        for img in range(2, imgs):
            load_x(img)
        conv(0, 1)
        for img in range(1, imgs):
            conv(img, 0)
            conv(img, 1)
    nc.compile()
    return nc


BATCH, H, W = 32, 56, 56
N_CORES = 8
IMGS = BATCH // N_CORES
_NC_CACHE = {}


def _get_nc():
    key = ("boxsum", IMGS, H, W)
    if key not in _NC_CACHE:
        _NC_CACHE[key] = _build_boxsum_nc(IMGS, H, W)
    return _NC_CACHE[key]


def _get_conv_nc():
    key = ("conv", IMGS, H, W)
    if key not in _NC_CACHE:
        _NC_CACHE[key] = _build_conv_nc(IMGS, H, W, hchunk=8, psum_bufs=7,
                                        gsz=4, tp_bufs=1)
    return _NC_CACHE[key]


def kernel(**inputs) -> np.ndarray:
    from concourse.bass_utils import run_bass_kernel_spmd

    x = np.ascontiguousarray(np.asarray(inputs["x"], dtype=np.float32))
    weight = np.ascontiguousarray(np.asarray(inputs["weight"], dtype=np.float32))
    assert x.shape == (BATCH, IN_C, H, W), x.shape
    assert weight.shape == (OUT_C * CKK, 1), weight.shape

    # sign(w) == +1 for all w >= 0 (the module init is rand()*1e-3); the
    # box-sum kernel is exact in that regime.  Any negative weight falls
    # back to the general dense-conv kernel.
    nc = _get_nc() if bool(np.all(weight >= 0)) else _get_conv_nc()
    in_maps = [
        {"x": x[c * IMGS:(c + 1) * IMGS], "w": weight}
        for c in range(N_CORES)
    ]
    res = run_bass_kernel_spmd(nc, in_maps, core_ids=list(range(N_CORES)))
    return np.concatenate([res.results[c]["y"] for c in range(N_CORES)], axis=0)
